# revision 1
# baseline (speedup 1.0000x reference)
"""Trainium2 Bass kernel for nn_CausalSelfAttention_49572512530497.

Sparse attention (local 256-window causal + strided-64 global, GQA 16q/4kv,
RoPE, sigmoid head gating) with fused projections, for B=2, S=2048, DIM=2048.

Sharding: 8 cores = 2 batches x 4 kv-head groups. Core c=(b,g) computes the
full pipeline for batch b and q-heads [4g, 4g+4) (which share kv head g), and
produces the partial output  attn_heads @ Wo.T[rows 512g:512(g+1)] of shape
[S, DIM].  The host sums the 4 per-group partials of each batch.

Instruction-count-oriented design (per-op floors dominate on TRN2):
 - projections / scores / output matmuls in float32r (full PE rate, fp32 data)
 - additive window mask applied by PE (identity @ mask accumulated into PSUM)
 - local + global scores share one PSUM bank; one exp each with accum_out
 - softmax normalization (and the 0.7/0.3 mix weights) folded into the
   P-transpose via a diag(w/l) moving operand built by GPSIMD
 - AV in bf16; all 4 heads accumulate into one PSUM bank per q-tile
 - RoPE multiplies on DVE (PSUM readers), adds on GPSIMD
"""

import numpy as np

import concourse.bass as bass
import concourse.mybir as mybir
import concourse.tile as tile
from concourse import bacc
from concourse.bass_utils import run_bass_kernel_spmd

B, S, DIM = 2, 2048, 2048
NH, NKV = 16, 4
HD = DIM // NH            # 128
GQ = NH // NKV            # 4 q-heads per kv head / per core
BASE = 10000.0
WINDOW, STRIDE = 256, 64
NG = S // STRIDE          # 32 global keys
SCALE = 1.0 / float(np.sqrt(HD))
NQT = S // 128            # 16 query tiles
NKC = DIM // 128          # 16 contraction chunks
NST = 4                   # seq strips for projections
STRIP = S // NST          # 512
MASKVAL = -1e30

f32 = mybir.dt.float32
f32r = mybir.dt.float32r
bf16 = mybir.dt.bfloat16
EXP = mybir.ActivationFunctionType.Exp
SIGMOID = mybir.ActivationFunctionType.Sigmoid


def _rope_tables():
    half = HD // 2
    inv_freq = 1.0 / (BASE ** (np.arange(0, half, dtype=np.float64) * 2.0 / HD))
    t = np.arange(S, dtype=np.float64)
    freqs = t[:, None] * inv_freq[None, :]          # [S, 64]
    cosT = np.cos(freqs).T.astype(np.float32)       # [64, S]
    sinT = np.sin(freqs).T.astype(np.float32)
    cos2 = np.concatenate([cosT, cosT], axis=0)     # [128, S]
    sin2s = np.concatenate([-sinT, sinT], axis=0)   # [128, S]
    return cos2, sin2s


def _win(qt):
    q0 = qt * 128
    wstart = max(0, q0 - WINDOW)
    return wstart, q0 + 128 - wstart


def _mask(qt):
    q0 = qt * 128
    wstart, w = _win(qt)
    qi = np.arange(128)[:, None] + q0
    kj = np.arange(w)[None, :] + wstart
    allowed = (kj <= qi) & (kj >= qi - WINDOW)
    return np.where(allowed, 0.0, MASKVAL).astype(np.float32)


def _build_nc():
    nc = bacc.Bacc()

    xt_d = nc.dram_tensor("xt", [DIM, S], f32r, kind="ExternalInput")
    wq_d = nc.dram_tensor("wq", [NKC, 128, GQ * 128], f32r, kind="ExternalInput")
    wkv_d = nc.dram_tensor("wkv", [NKC, 128, 256], f32r, kind="ExternalInput")
    wr_d = nc.dram_tensor("wr", [NKC, 128, GQ], f32r, kind="ExternalInput")
    br_d = nc.dram_tensor("br", [GQ, 1], f32, kind="ExternalInput")
    wo_d = nc.dram_tensor("wo", [GQ, 128, DIM], f32r, kind="ExternalInput")
    out_d = nc.dram_tensor("out", [S, DIM], f32, kind="ExternalOutput")

    cos2_np, sin2s_np = _rope_tables()
    cos2_d = nc.inline_tensor(cos2_np, "cos2c")
    sin2s_d = nc.inline_tensor(sin2s_np, "sin2sc")
    kj = np.arange(128)[:, None]
    qi = np.arange(128)[None, :]
    mlo = np.where(kj >= qi, 0.0, MASKVAL).astype(np.float32)   # first window chunk
    mhi = np.where(kj <= qi, 0.0, MASKVAL).astype(np.float32)   # diagonal chunk
    mlo_d = nc.inline_tensor(np.tile(mlo, (1, GQ)), "mloc")     # [128, 512]
    mhi_d = nc.inline_tensor(np.tile(mhi, (1, GQ)), "mhic")
    eye = np.eye(128)
    idf_d = nc.inline_tensor(eye.astype(np.float32), "idfc")
    import ml_dtypes
    w07_d = nc.inline_tensor(np.full((128, 1), 1.0 / 0.7, ml_dtypes.bfloat16),
                             "w07c")
    w03_d = nc.inline_tensor(np.full((128, 1), 1.0 / 0.3, ml_dtypes.bfloat16),
                             "w03c")
    ones1_d = nc.inline_tensor(np.ones((128, 128), np.float32), "ones1c")

    with tile.TileContext(nc) as tc:
        with tc.tile_pool(name="glob", bufs=1) as glob:
            qT = glob.tile([128, GQ * S], f32r, tag="qTa", name="qTa")
            qTh_view = qT.rearrange("p (h s) -> p h s", h=GQ)
            kT = glob.tile([128, S], f32r, tag="kT", name="kT")
            vT = glob.tile([128, S], f32, tag="vT", name="vT")
            v_bf = glob.tile([128, S], bf16, tag="v_bf", name="v_bf")
            vg_bf = glob.tile([32, 128], bf16, tag="vgbf", name="vgbf")
            kg = glob.tile([128, NG], f32r, tag="kg", name="kg")
            gateS = glob.tile([GQ, S], f32r, tag="gateS", name="gateS")
            gAB = [glob.tile([65, S], f32r, tag=f"gAB{i}", name=f"gAB{i}")
                   for i in range(2)]
            def _grow(h, sl=slice(None)):
                return gAB[h // 2][(h % 2) * 64:(h % 2) * 64 + 1, sl]
            cos2 = glob.tile([128, S], f32, tag="cos2", name="cos2")
            sin2s = glob.tile([128, S], f32, tag="sin2s", name="sin2s")
            m_lo = glob.tile([128, 512], f32r, tag="m_lo", name="m_lo")
            m_hi = glob.tile([128, 512], f32r, tag="m_hi", name="m_hi")
            id_f = glob.tile([128, 128], f32, tag="idf", name="idf")
            id_r = glob.tile([128, 128], f32r, tag="idr", name="idr")
            w07_bf = glob.tile([128, 1], bf16, tag="w07bf", name="w07bf")
            w03_bf = glob.tile([128, 1], bf16, tag="w03bf", name="w03bf")
            ones1_r = glob.tile([128, 128], f32r, tag="ones1r", name="ones1r")
            br_t = glob.tile([GQ, 1], f32, tag="br", name="br")

            nc.sync.dma_start(out=br_t, in_=br_d[:, :])
            nc.sync.dma_start(out=cos2, in_=cos2_d[:, :])
            nc.sync.dma_start(out=sin2s, in_=sin2s_d[:, :])
            nc.sync.dma_start(out=ones1_r, in_=ones1_d[:, :].bitcast(f32r))

            # ================= phase 1: fused projections =================
            with tc.tile_pool(name="wts", bufs=1) as wpool, \
                 tc.tile_pool(name="xs", bufs=12) as xpool, \
                 tc.tile_pool(name="pps", bufs=1, space="PSUM") as ppool, \
                 tc.tile_pool(name="ptmp", bufs=4) as tpool:
                wq_sb = [wpool.tile([128, GQ * 128], f32r, tag=f"wq{k}",
                                    name=f"wq{k}") for k in range(NKC)]
                wkv_sb = [wpool.tile([128, 256], f32r, tag=f"wkv{k}",
                                     name=f"wkv{k}") for k in range(NKC)]
                wr_sb = [wpool.tile([128, GQ], f32r, tag=f"wr{k}", name=f"wr{k}")
                         for k in range(NKC)]

                for st in range(NST):
                    sl = slice(st * STRIP, (st + 1) * STRIP)
                    q_ps = [ppool.tile([128, STRIP], f32, tag=f"qps{d}",
                                       name=f"qps{d}") for d in range(GQ)]
                    kv_ps = [ppool.tile([128, STRIP], f32, tag=f"kvps{d}",
                                        name=f"kvps{d}") for d in range(2)]
                    g_ps = ppool.tile([GQ, STRIP], f32, tag="gps2", name="gps", bufs=1)
                    for k in range(NKC):
                        xk = xpool.tile([128, STRIP], f32r, tag="xk", name="xk")
                        nc.sync.dma_start(
                            out=xk, in_=xt_d[k * 128:(k + 1) * 128, sl])
                        if st == 0:
                            nc.sync.dma_start(out=wq_sb[k], in_=wq_d[k])
                            nc.sync.dma_start(out=wkv_sb[k], in_=wkv_d[k])
                            nc.sync.dma_start(out=wr_sb[k], in_=wr_d[k])
                        mmargs = dict(start=(k == 0), stop=(k == NKC - 1))
                        for d in range(GQ):
                            nc.tensor.matmul(
                                q_ps[d], wq_sb[k][:, d * 128:(d + 1) * 128],
                                xk, **mmargs)
                        for d in range(2):
                            nc.tensor.matmul(
                                kv_ps[d], wkv_sb[k][:, d * 128:(d + 1) * 128],
                                xk, **mmargs)
                        nc.tensor.matmul(g_ps, wr_sb[k], xk, **mmargs)

                    # gate first: sigmoid rows, then re-base each head's row
                    # to a matmul-legal start partition via tiny SBUF DMAs
                    nc.scalar.activation(gateS[:, sl], g_ps, SIGMOID,
                                         bias=br_t, scale=1.0)
                    for h in range(GQ):
                        nc.sync.dma_start(out=_grow(h, sl),
                                          in_=gateS[h:h + 1, sl])
                    # RoPE evacuation: out = ps*cos2 + swap(ps)*[-sin; sin]
                    # then the per-query sigmoid gate is folded into q via a
                    # PE broadcast of the gate row (ones1 @ gate_row -> PSUM)
                    for h in range(GQ):
                        ps = q_ps[h]
                        qsl = qTh_view[:, h, sl]
                        a_ps = ppool.tile([128, STRIP], f32, tag="gps",
                                          name="a_ps", bufs=1)
                        base = (h % 2) * 64
                        nc.tensor.matmul(a_ps, ones1_r[base:base + 1, :],
                                         _grow(h, sl),
                                         start=True, stop=True)
                        # RoPE from PSUM (swapped-half reads are PSUM-side),
                        # gate applied last from the broadcast PSUM row
                        tmp = tpool.tile([128, STRIP], f32, tag="ropetmp",
                                         name="ropetmp")
                        nc.vector.tensor_mul(tmp[0:64], ps[64:128],
                                             sin2s[0:64, sl])
                        nc.vector.tensor_mul(tmp[64:128], ps[0:64],
                                             sin2s[64:128, sl])
                        nc.vector.tensor_mul(qsl, ps, cos2[:, sl])
                        nc.gpsimd.tensor_add(qsl, qsl, tmp)
                        nc.vector.tensor_mul(qsl, qsl, a_ps)
                    ps = kv_ps[0]
                    tmp = tpool.tile([128, STRIP], f32, tag="ropetmp",
                                     name="ropetmp")
                    nc.vector.tensor_mul(tmp[0:64], ps[64:128], sin2s[0:64, sl])
                    nc.vector.tensor_mul(tmp[64:128], ps[0:64], sin2s[64:128, sl])
                    nc.vector.tensor_mul(kT[:, sl], ps, cos2[:, sl])
                    nc.gpsimd.tensor_add(kT[:, sl], kT[:, sl], tmp)
                    nc.scalar.copy(vT[:, sl], kv_ps[1])
            # ========= phase 1b: v transposes, global k/v =========
            nc.sync.dma_start(out=id_f, in_=idf_d[:, :])
            nc.sync.dma_start(out=id_r, in_=idf_d[:, :].bitcast(f32r))
            nc.sync.dma_start(out=m_lo, in_=mlo_d[:, :].bitcast(f32r))
            nc.sync.dma_start(out=m_hi, in_=mhi_d[:, :].bitcast(f32r))
            nc.sync.dma_start(out=w07_bf, in_=w07_d[:, :])
            nc.sync.dma_start(out=w03_bf, in_=w03_d[:, :])
            with tc.tile_pool(name="vtps", bufs=2, space="PSUM") as vpp, \
                 tc.tile_pool(name="tps", bufs=2) as tp2:
                # v transposes: 4 per PSUM bank, 4 wide evacuations
                for grp in range(4):
                    vp = vpp.tile([128, 512], f32, tag="vtp", name="vtp")
                    for j in range(4):
                        c = grp * 4 + j
                        nc.tensor.transpose(vp[:, j * 128:(j + 1) * 128],
                                            vT[:, c * 128:(c + 1) * 128], id_f)
                    dst = v_bf[:, grp * 512:(grp + 1) * 512]
                    if grp % 2 == 0:
                        nc.scalar.copy(dst, vp)
                    else:
                        nc.vector.tensor_copy(dst, vp)
                # dense copies of the strided global k/v slices
                vgs = tp2.tile([128, NG], f32, tag="vgs", name="vgs")
                nc.scalar.copy(vgs, vT[:, 0:S:STRIDE])
                nc.scalar.copy(kg, kT[:, 0:S:STRIDE])
                vgp = vpp.tile([32, 128], f32, tag="vgtp", name="vgtp", bufs=1)
                nc.tensor.transpose(vgp, vgs, id_f)
                nc.scalar.copy(vg_bf, vgp)

            # ============ phase 2: attention + output projection ============
            # S^T orientation: scores come out pre-transposed, all 4 GQA heads
            # wide (N=512).  Row sums via ones-matmuls; per-query normalization
            # and the 0.7/0.3 mix applied post-AV with PE-broadcast 1/l rows.
            with tc.tile_pool(name="wow", bufs=1) as wop, \
                 tc.tile_pool(name="att", bufs=4) as apool, \
                 tc.tile_pool(name="atts", bufs=2) as spool, \
                 tc.tile_pool(name="outp", bufs=4) as opool, \
                 tc.tile_pool(name="ps_s", bufs=3, space="PSUM") as pss, \
                 tc.tile_pool(name="ps_l", bufs=1, space="PSUM") as psl, \
                 tc.tile_pool(name="ps_av", bufs=2, space="PSUM") as psav, \
                 tc.tile_pool(name="ps_wo", bufs=2, space="PSUM") as pswo:
                woT = [wop.tile([128, DIM], f32r, tag=f"wo{h}", name=f"wo{h}")
                       for h in range(GQ)]
                for h in range(GQ):
                    nc.sync.dma_start(out=woT[h], in_=wo_d[h])

                for qt in range(NQT):
                    q0 = qt * 128
                    wstart, w = _win(qt)
                    nch = w // 128
                    qrhs = qTh_view[:, :, q0:q0 + 128]        # [128, GQ, 128]
                    l_ps = psl.tile([64, 512], f32, tag="lps", name="lps")
                    # ---- local chunks: S^T, mask, exp, l, AV ----
                    av_l = psav.tile([128, 512], f32, tag="av", name="av_l")
                    pTs = []
                    for c in range(nch):
                        kc = wstart // 128 + c
                        ksl = slice(kc * 128, (kc + 1) * 128)
                        sp = pss.tile([128, 512], f32, tag="sps", name="sps")
                        last = (qt == 0) or (c == nch - 1) or (qt >= 2 and c == 0)
                        nc.tensor.matmul(sp, kT[:, ksl], qrhs,
                                         start=True, stop=not last)
                        if qt >= 2 and c == 0:
                            nc.tensor.matmul(sp, id_r, m_lo, start=False,
                                             stop=True)
                        elif c == nch - 1:
                            nc.tensor.matmul(sp, id_r, m_hi, start=False,
                                             stop=True)
                        pT = apool.tile([128, 512], bf16, tag="pT", name="pT")
                        nc.scalar.activation(pT, sp, EXP, scale=SCALE)
                        nc.tensor.matmul(l_ps[0:1, :], w07_bf, pT,
                                         start=(c == 0), stop=(c == nch - 1))
                        nc.tensor.matmul(av_l, v_bf[:, ksl], pT,
                                         start=(c == 0), stop=(c == nch - 1))
                        pTs.append(pT)
                    # ---- global: S^T_g, exp, l_g, AV_g ----
                    spg = pss.tile([32, 512], f32, tag="sps", name="spg")
                    nc.tensor.matmul(spg, kg, qrhs, start=True, stop=True)
                    pTg = apool.tile([32, 512], bf16, tag="pTg", name="pTg")
                    nc.scalar.activation(pTg, spg, EXP, scale=SCALE)
                    nc.tensor.matmul(l_ps[32:33, :], w03_bf[0:32, :], pTg,
                                     start=True, stop=True)
                    av_g = psav.tile([128, 512], f32, tag="av", name="av_g")
                    nc.tensor.matmul(av_g, vg_bf, pTg, start=True, stop=True)
                    # ---- normalization + 0.7/0.3 mix ----
                    r_l = spool.tile([1, 512], f32r, tag="r_l", name="r_l")
                    r_g = spool.tile([1, 512], f32r, tag="r_g", name="r_g")
                    with nc.allow_low_precision("f32r == f32 bits"):
                        nc.vector.reciprocal(r_l, l_ps[0:1, :])
                        nc.vector.reciprocal(r_g, l_ps[32:33, :])
                    rbp_l = pss.tile([128, 512], f32, tag="sps", name="rbp_l")
                    nc.tensor.matmul(rbp_l, ones1_r[0:1, :], r_l,
                                     start=True, stop=True)
                    rbp_g = pss.tile([128, 512], f32, tag="sps", name="rbp_g")
                    nc.tensor.matmul(rbp_g, ones1_r[0:1, :], r_g,
                                     start=True, stop=True)
                    rb_l = spool.tile([128, 512], f32, tag="rb_l", name="rb_l")
                    rb_g = spool.tile([128, 512], f32, tag="rb_g", name="rb_g")
                    nc.scalar.copy(rb_l, rbp_l)
                    nc.vector.tensor_copy(rb_g, rbp_g)
                    t_l = spool.tile([128, 512], f32, tag="t_l", name="t_l")
                    t_g = spool.tile([128, 512], f32, tag="t_g", name="t_g")
                    nc.vector.tensor_mul(t_l, av_l, rb_l)
                    nc.vector.tensor_mul(t_g, av_g, rb_g)
                    at_all = spool.tile([128, 512], f32r, tag="at", name="at", bufs=3)
                    nc.gpsimd.tensor_add(at_all, t_l, t_g)
                    # ---- output projection for this q tile ----
                    for os_ in range(4):
                        osl = slice(os_ * 512, (os_ + 1) * 512)
                        wo_ps = pswo.tile([128, 512], f32, tag="wops", name="wops")
                        for h in range(GQ):
                            nc.tensor.matmul(wo_ps,
                                             at_all[:, h * 128:(h + 1) * 128],
                                             woT[h][:, osl],
                                             start=(h == 0), stop=(h == GQ - 1))
                        ot = opool.tile([128, 512], f32, tag="ot", name="ot")
                        if os_ % 2 == 0:
                            nc.scalar.copy(ot, wo_ps)
                        else:
                            nc.vector.tensor_copy(ot, wo_ps)
                        nc.sync.dma_start(out=out_d[q0:q0 + 128, osl], in_=ot)

    nc.finalize()
    return nc


_NC_CACHE = {}


def _get_nc():
    if "nc" not in _NC_CACHE:
        _NC_CACHE["nc"] = _build_nc()
    return _NC_CACHE["nc"]


def _prep_core_inputs(x, Wq, Wkv, Wo, Wr, br, b, g):
    xt = np.ascontiguousarray(x[b].T).astype(np.float32)           # [DIM, S]
    wq_slice = Wq[g * GQ * HD:(g + 1) * GQ * HD, :]                # [512, DIM]
    wq_t = np.ascontiguousarray(
        wq_slice.T.reshape(NKC, 128, GQ * 128)).astype(np.float32)
    krow = Wkv[g * HD:(g + 1) * HD, :]                             # [128, DIM]
    vrow = Wkv[NKV * HD + g * HD: NKV * HD + (g + 1) * HD, :]      # [128, DIM]
    kv = np.concatenate([krow, vrow], axis=0)                      # [256, DIM]
    wkv_t = np.ascontiguousarray(
        kv.T.reshape(NKC, 128, 256)).astype(np.float32)
    wr_slice = Wr[g * GQ:(g + 1) * GQ, :]                          # [4, DIM]
    wr_t = np.ascontiguousarray(wr_slice.T.reshape(NKC, 128, GQ)).astype(np.float32)
    br_s = np.ascontiguousarray(
        br[g * GQ:(g + 1) * GQ].reshape(GQ, 1)).astype(np.float32)
    wo_t = np.ascontiguousarray(
        Wo[:, g * GQ * HD:(g + 1) * GQ * HD].T.reshape(GQ, 128, DIM)
    ).astype(np.float32)
    return {"xt": xt, "wq": wq_t, "wkv": wkv_t, "wr": wr_t, "br": br_s,
            "wo": wo_t}


def kernel(x, Wq, Wkv, Wo, Wr, br):
    x = np.asarray(x, dtype=np.float32)
    Wq = np.asarray(Wq, dtype=np.float32)
    Wkv = np.asarray(Wkv, dtype=np.float32)
    Wo = np.asarray(Wo, dtype=np.float32)
    Wr = np.asarray(Wr, dtype=np.float32)
    br = np.asarray(br, dtype=np.float32)

    nc = _get_nc()
    in_maps = []
    for c in range(8):
        b, g = divmod(c, NKV)
        in_maps.append(_prep_core_inputs(x, Wq, Wkv, Wo, Wr, br, b, g))
    res = run_bass_kernel_spmd(nc, in_maps, list(range(8)))
    out = np.zeros((B, S, DIM), dtype=np.float32)
    for c in range(8):
        b, g = divmod(c, NKV)
        out[b] += res.results[c]["out"]
    return out

